# revision 7
# baseline (speedup 1.0000x reference)
"""Haar DWT (2x2 stride-2 block decomposition) on 8 Trainium2 NeuronCores.

Input x: (32, 3, 512, 512) f32. Outputs (ll, lh, hl, hh): each (32, 3, 256, 256).

Sharding: pure data parallel over the batch dim — 4 images per core, viewed as
12 channel images of 512x512 per core, one channel per iteration.

The vertical (row-pair) butterfly runs on the TensorEngine: a constant 128x128
weight matrix W maps 128 image rows to 64 halved row-sums (partitions 0..63)
and 64 halved row-diffs (partitions 64..127) in one matmul per 128-row tile
(4 per channel). The weights are +-0.5 (exact powers of two) and all other
entries are exactly zero, so the result is bit-identical to the fp32 two-op
formulation. The horizontal stride-2 column combine is then just 2 DVE ops per
tile — (even+odd) producing ll|lh stacked over partitions, and (odd-even)
producing hl|hh — reading PSUM, writing a stacked SBUF tile stored with one
fully contiguous 1 MB DMA per channel.

ACT does no elementwise work and issues the store DMAs on the second HWDGE
ring, overlapping the load ring on Sync.
"""

import sys

import numpy as np

if "/opt/trn_rl_repo" not in sys.path:
    sys.path.insert(0, "/opt/trn_rl_repo")

from concourse import bacc, bass, mybir
from concourse import tile
from concourse.bass_utils import run_bass_kernel_spmd

N_CORES = 8
B, C, H, W = 32, 3, 512, 512
BPC = B // N_CORES  # images per core
NCH = BPC * C  # channel images per core (12)
P = 128  # SBUF partitions
NT = H // P  # 128-row tiles per channel (4)
HW_OUT = H // 2  # 256

_CACHE = {}


def _butterfly_weights():
    """W[k, m]: m<64 -> 0.5*(row 2m + row 2m+1); m>=64 -> 0.5*(row 2m'+1 - row 2m')."""
    w = np.zeros((P, P), dtype=np.float32)
    for m in range(64):
        w[2 * m, m] = 0.5
        w[2 * m + 1, m] = 0.5
        w[2 * m, 64 + m] = -0.5
        w[2 * m + 1, 64 + m] = 0.5
    return w


def _build():
    nc = bacc.Bacc("TRN2", target_bir_lowering=False, debug=False)
    f32 = mybir.dt.float32
    # x viewed as [NCH, tile, row-in-tile, W]
    x = nc.dram_tensor("x", [NCH, NT, P, W], f32, kind="ExternalInput")
    w = nc.dram_tensor("w", [P, P], f32, kind="ExternalInput")
    # out[ch, p, t, g, j]: p<64,g=0: ll row 64t+p | p>=64,g=0: lh row 64t+p-64
    #                      p<64,g=1: hl          | p>=64,g=1: hh
    out = nc.dram_tensor("out", [NCH, P, NT, 2, HW_OUT], f32, kind="ExternalOutput")
    xa = x.ap()
    oa = out.ap()
    with tile.TileContext(nc) as tc:
        with (
            tc.tile_pool(name="p", bufs=5) as pool,
            tc.tile_pool(name="w", bufs=1) as wpool,
            tc.tile_pool(name="ps", bufs=8, space=bass.MemorySpace.PSUM) as psum,
        ):
            wt = wpool.tile([P, P], f32)
            nc.sync.dma_start(out=wt[:], in_=w.ap())
            for i in range(NCH):
                # per-tile loads (256 KB each) so matmul t can start as soon
                # as its 128 rows have landed, not after the whole channel
                xts = []
                for t in range(NT):
                    xt = pool.tile([P, W], f32, tag=f"x{t}")
                    nc.sync.dma_start(out=xt[:], in_=xa[i, t])
                    xts.append(xt)
                outt = pool.tile([P, NT, 2, HW_OUT], f32)
                for t in range(NT):
                    pt = psum.tile([P, W], f32)
                    nc.tensor.matmul(pt[:], wt[:], xts[t][:], start=True, stop=True)
                    pv = pt[:].rearrange("p (j two) -> p j two", two=2)
                    # DVE can read at most one PSUM operand per instruction:
                    # ACT (otherwise idle) stages the even columns into SBUF.
                    cp = pool.tile([P, HW_OUT], f32)
                    nc.scalar.copy(cp[:], pv[:, :, 0])
                    nc.vector.tensor_add(outt[:, t, 0], pv[:, :, 1], cp[:])
                    nc.vector.tensor_sub(outt[:, t, 1], pv[:, :, 1], cp[:])
                # half-channel stores (512 KB) to stream the tail out earlier
                nc.scalar.dma_start(out=oa[i, :, 0:2], in_=outt[:, 0:2])
                nc.scalar.dma_start(out=oa[i, :, 2:4], in_=outt[:, 2:4])
    nc.compile()
    return nc


def _get_nc():
    if "nc" not in _CACHE:
        _CACHE["nc"] = _build()
    return _CACHE["nc"]


def run(x, **spmd_kwargs):
    """Run the DWT on 8 cores; returns (results_tuple, BassKernelResults)."""
    nc = _get_nc()
    xs = np.ascontiguousarray(np.asarray(x, dtype=np.float32)).reshape(
        N_CORES, NCH, NT, P, W
    )
    wmat = _butterfly_weights()
    in_maps = [{"x": xs[i], "w": wmat} for i in range(N_CORES)]
    res = run_bass_kernel_spmd(nc, in_maps, core_ids=list(range(N_CORES)), **spmd_kwargs)
    # per-core out: (NCH, P, NT, 2, HW_OUT)
    full = np.stack([res.results[i]["out"] for i in range(N_CORES)])
    # -> (cores, NCH, NT, P, 2, j): out image row r = 64*t + (p mod 64)
    full = full.transpose(0, 1, 3, 2, 4, 5)
    def expand(sl):  # (cores, NCH, NT, 64, j) -> (B, C, 256, 256)
        return np.ascontiguousarray(sl).reshape(B, C, HW_OUT, HW_OUT)
    ll = expand(full[:, :, :, 0:64, 0, :])
    lh = expand(full[:, :, :, 64:128, 0, :])
    hl = expand(full[:, :, :, 0:64, 1, :])
    hh = expand(full[:, :, :, 64:128, 1, :])
    return (ll, lh, hl, hh), res


def kernel(x):
    out, _ = run(x)
    return out


# revision 8
# speedup vs baseline: 1.1455x; 1.1455x over previous
"""Haar DWT (2x2 stride-2 block decomposition) on 8 Trainium2 NeuronCores.

Input x: (32, 3, 512, 512) f32. Outputs (ll, lh, hl, hh): each (32, 3, 256, 256).

Sharding: pure data parallel over the batch dim — 4 images per core, viewed as
12 channel images of 512x512 per core, one channel per iteration.

The vertical (row-pair) butterfly runs on the TensorEngine: a constant 128x128
weight matrix W maps 128 image rows to 64 halved row-sums (partitions 0..63)
and 64 halved row-diffs (partitions 64..127) in one matmul per 128-row tile
(4 per channel). The weights are +-0.5 (exact powers of two) and all other
entries are exactly zero, so the result is bit-identical to the fp32 two-op
formulation. The horizontal stride-2 column combine is then just 2 DVE ops per
tile — (even+odd) producing ll|lh stacked over partitions, and (odd-even)
producing hl|hh — reading PSUM, writing a stacked SBUF tile stored with one
fully contiguous 1 MB DMA per channel.

ACT does no elementwise work and issues the store DMAs on the second HWDGE
ring, overlapping the load ring on Sync.
"""

import sys

import numpy as np

if "/opt/trn_rl_repo" not in sys.path:
    sys.path.insert(0, "/opt/trn_rl_repo")

from concourse import bacc, bass, mybir
from concourse import tile
from concourse.bass_utils import run_bass_kernel_spmd

N_CORES = 8
B, C, H, W = 32, 3, 512, 512
BPC = B // N_CORES  # images per core
NCH = BPC * C  # channel images per core (12)
P = 128  # SBUF partitions
NT = H // P  # 128-row tiles per channel (4)
HW_OUT = H // 2  # 256

_CACHE = {}


def _butterfly_weights():
    """W[k, m]: m<64 -> 0.5*(row 2m + row 2m+1); m>=64 -> 0.5*(row 2m'+1 - row 2m')."""
    w = np.zeros((P, P), dtype=np.float32)
    for m in range(64):
        w[2 * m, m] = 0.5
        w[2 * m + 1, m] = 0.5
        w[2 * m, 64 + m] = -0.5
        w[2 * m + 1, 64 + m] = 0.5
    return w


def _build():
    nc = bacc.Bacc("TRN2", target_bir_lowering=False, debug=False)
    f32 = mybir.dt.float32
    # x viewed as [NCH, tile, row-in-tile, W]
    x = nc.dram_tensor("x", [NCH, NT, P, W], f32, kind="ExternalInput")
    w = nc.dram_tensor("w", [P, P], f32, kind="ExternalInput")
    # out[ch, p, t, g, j]: p<64,g=0: ll row 64t+p | p>=64,g=0: lh row 64t+p-64
    #                      p<64,g=1: hl          | p>=64,g=1: hh
    out = nc.dram_tensor("out", [NCH, P, NT, 2, HW_OUT], f32, kind="ExternalOutput")
    xa = x.ap()
    oa = out.ap()
    with tile.TileContext(nc) as tc:
        with (
            tc.tile_pool(name="p", bufs=5) as pool,
            tc.tile_pool(name="w", bufs=1) as wpool,
            tc.tile_pool(name="ps", bufs=8, space=bass.MemorySpace.PSUM) as psum,
        ):
            wt = wpool.tile([P, P], f32)
            nc.sync.dma_start(out=wt[:], in_=w.ap())
            for i in range(NCH):
                xin = pool.tile([P, NT, W], f32)
                if i == 0:
                    # split the first load so matmuls start ~4 us earlier
                    for t in range(NT):
                        nc.sync.dma_start(out=xin[:, t, :], in_=xa[i, t])
                else:
                    # (t, p, w) -> (p, t, w); fully sequential DRAM read
                    nc.sync.dma_start(out=xin[:], in_=xa[i].transpose([1, 0, 2]))
                outt = pool.tile([P, NT, 2, HW_OUT], f32)
                for t in range(NT):
                    pt = psum.tile([P, W], f32)
                    nc.tensor.matmul(pt[:], wt[:], xin[:, t, :], start=True, stop=True)
                    pv = pt[:].rearrange("p (j two) -> p j two", two=2)
                    # DVE can read at most one PSUM operand per instruction:
                    # ACT (otherwise idle) stages the even columns into SBUF.
                    cp = pool.tile([P, HW_OUT], f32)
                    nc.scalar.copy(cp[:], pv[:, :, 0])
                    nc.vector.tensor_add(outt[:, t, 0], pv[:, :, 1], cp[:])
                    nc.vector.tensor_sub(outt[:, t, 1], pv[:, :, 1], cp[:])
                if i == NCH - 1:
                    # split the last store so the tail drains in halves
                    nc.scalar.dma_start(out=oa[i, :, 0:2], in_=outt[:, 0:2])
                    nc.scalar.dma_start(out=oa[i, :, 2:4], in_=outt[:, 2:4])
                else:
                    nc.scalar.dma_start(out=oa[i], in_=outt[:])
    nc.compile()
    return nc


def _get_nc():
    if "nc" not in _CACHE:
        _CACHE["nc"] = _build()
    return _CACHE["nc"]


def run(x, **spmd_kwargs):
    """Run the DWT on 8 cores; returns (results_tuple, BassKernelResults)."""
    nc = _get_nc()
    xs = np.ascontiguousarray(np.asarray(x, dtype=np.float32)).reshape(
        N_CORES, NCH, NT, P, W
    )
    wmat = _butterfly_weights()
    in_maps = [{"x": xs[i], "w": wmat} for i in range(N_CORES)]
    res = run_bass_kernel_spmd(nc, in_maps, core_ids=list(range(N_CORES)), **spmd_kwargs)
    # per-core out: (NCH, P, NT, 2, HW_OUT)
    full = np.stack([res.results[i]["out"] for i in range(N_CORES)])
    # -> (cores, NCH, NT, P, 2, j): out image row r = 64*t + (p mod 64)
    full = full.transpose(0, 1, 3, 2, 4, 5)
    def expand(sl):  # (cores, NCH, NT, 64, j) -> (B, C, 256, 256)
        return np.ascontiguousarray(sl).reshape(B, C, HW_OUT, HW_OUT)
    ll = expand(full[:, :, :, 0:64, 0, :])
    lh = expand(full[:, :, :, 64:128, 0, :])
    hl = expand(full[:, :, :, 0:64, 1, :])
    hh = expand(full[:, :, :, 64:128, 1, :])
    return (ll, lh, hl, hh), res


def kernel(x):
    out, _ = run(x)
    return out
